# revision 9
# baseline (speedup 1.0000x reference)
"""ConsciousnessTransformer Trainium2 kernel.

Data-parallel over batch: 8 NeuronCores x 4 sequences each. Activations are
kept feature-major ([d, token]) in SBUF so every linear contracts over the
partition dim. Matmuls run in float32r (full-rate on PE, ~tf32 accuracy);
elementwise math stays fp32. LayerNorm stats and row->tile broadcasts use
ones-vector matmuls on the PE; attention softmax runs along the partition
axis with the normalizer folded in as a ones-column of V.
"""

import os
import sys

sys.path.insert(0, "/opt/trn_rl_repo")

import numpy as np

D = 512
DEPTH = 12
H = 8
HD = 64
V = 10000
B = 32
N = 512
FF = 2048
EPS = 1e-5
NCORES = 8
BC = B // NCORES  # sequences per core
NB = 50  # skills
NE = 25  # emotions

_PROG = None  # (nc, input_names)
LAST_RUN = None  # BassKernelResults of the most recent execution


# ----------------------------------------------------------------------------
# device program
# ----------------------------------------------------------------------------
def _build_program(depth):
    import concourse.bass as bass
    import concourse.tile as tile
    from concourse import bacc, mybir
    from concourse.masks import make_identity

    f32 = mybir.dt.float32
    f32r = mybir.dt.float32r
    i32 = mybir.dt.int32
    AF = mybir.ActivationFunctionType

    nc = bacc.Bacc("TRN2", target_bir_lowering=False, debug=False,
                   num_devices=NCORES)

    # ---- DRAM inputs -------------------------------------------------------
    xids_d = nc.dram_tensor("xids", [BC * 4, 128], i32, kind="ExternalInput")
    tok_d = nc.dram_tensor("tok", [V, D], f32, kind="ExternalInput")
    post_d = nc.dram_tensor("posT", [128, 4, N], f32, kind="ExternalInput")
    skemo_d = nc.dram_tensor("skemoT", [128, 4, BC], f32, kind="ExternalInput")

    wqk_d = nc.dram_tensor("wqk", [depth, 128, 4, 1024], f32r, kind="ExternalInput")
    wv_d = nc.dram_tensor("wv", [depth, 128, 4, 512], f32r, kind="ExternalInput")
    wproj_d = nc.dram_tensor("wproj", [depth, 128, 4, 512], f32r, kind="ExternalInput")
    wfc1_d = nc.dram_tensor("wfc1", [depth, 128, 4, FF], f32r, kind="ExternalInput")
    wfc2_d = nc.dram_tensor("wfc2", [depth, 128, 16, 512], f32r, kind="ExternalInput")

    qkb_d = nc.dram_tensor("qkb", [depth, 128, 8], f32, kind="ExternalInput")
    vb_d = nc.dram_tensor("vb", [depth, 128, 4], f32, kind="ExternalInput")
    projb_d = nc.dram_tensor("projb", [depth, 128, 4], f32, kind="ExternalInput")
    fc1b_d = nc.dram_tensor("fc1b", [depth, 128, 16], f32, kind="ExternalInput")
    fc2b_d = nc.dram_tensor("fc2b", [depth, 128, 4], f32, kind="ExternalInput")
    ln1w_d = nc.dram_tensor("ln1w", [depth, 128, 4], f32, kind="ExternalInput")
    ln1b_d = nc.dram_tensor("ln1b", [depth, 128, 4], f32, kind="ExternalInput")
    ln2w_d = nc.dram_tensor("ln2w", [depth, 128, 4], f32, kind="ExternalInput")
    ln2b_d = nc.dram_tensor("ln2b", [depth, 128, 4], f32, kind="ExternalInput")
    normw_d = nc.dram_tensor("normw", [128, 4], f32, kind="ExternalInput")
    normb_d = nc.dram_tensor("normb", [128, 4], f32, kind="ExternalInput")

    # heads: (name, O1, act1, O2)
    HEADS = [
        ("sk", 1024, AF.Gelu, NB),
        ("em", 512, AF.Gelu, NE),
        ("co", 512, AF.Gelu, 1),
        ("cr", 1024, AF.Gelu, 512),
        ("wi", 512, AF.Tanh, 256),
        ("im", 512, AF.Gelu, 512),
    ]
    head_dram = {}
    for nm, O1, _, O2 in HEADS:
        O2p = O2 + (O2 % 2)
        head_dram[nm] = (
            nc.dram_tensor(f"{nm}1w", [128, 4, O1], f32r, kind="ExternalInput"),
            nc.dram_tensor(f"{nm}1b", [128, O1 // 128], f32, kind="ExternalInput"),
            nc.dram_tensor(f"{nm}2w", [128, O1 // 128, O2p], f32r, kind="ExternalInput"),
            nc.dram_tensor(f"{nm}2bt", [BC, O2], f32, kind="ExternalInput"),
        )

    # ---- DRAM outputs ------------------------------------------------------
    pooled_d = nc.dram_tensor("pooled", [BC, D], f32, kind="ExternalOutput")
    out_head_d = {
        "sk": nc.dram_tensor("skills", [BC, NB], f32, kind="ExternalOutput"),
        "em": nc.dram_tensor("emotions", [BC, NE], f32, kind="ExternalOutput"),
        "co": nc.dram_tensor("consc", [BC, 1], f32, kind="ExternalOutput"),
        "cr": nc.dram_tensor("creat", [BC, 512], f32, kind="ExternalOutput"),
        "wi": nc.dram_tensor("wisdom", [BC, 256], f32, kind="ExternalOutput"),
        "im": nc.dram_tensor("improv", [BC, 512], f32, kind="ExternalOutput"),
    }

    with tile.TileContext(nc) as tc:
        with (
            tc.tile_pool(name="const", bufs=1) as cpool,
            tc.tile_pool(name="hres", bufs=1) as hpool,
            tc.tile_pool(name="w", bufs=1) as wpool,
            tc.tile_pool(name="wb", bufs=2) as wbpool,
            tc.tile_pool(name="bias", bufs=1) as bpool,
            tc.tile_pool(name="big", bufs=6) as big,
            tc.tile_pool(name="expp", bufs=5) as expp,
            tc.tile_pool(name="u", bufs=4) as upool,
            tc.tile_pool(name="rows", bufs=1) as rpool,
            tc.tile_pool(name="ps_mm", bufs=3, space="PSUM") as ps_mm,
            tc.tile_pool(name="ps_sc", bufs=2, space="PSUM") as ps_sc,
            tc.tile_pool(name="ps_o", bufs=1, space="PSUM") as ps_o,
            tc.tile_pool(name="ps_st", bufs=2, space="PSUM") as ps_st,
        ):
            # ---- constants -----------------------------------------------
            ident = cpool.tile([128, 128], f32)
            make_identity(nc, ident[:])
            ones_col_f = cpool.tile([128, 1], f32)
            nc.vector.memset(ones_col_f[:], 1.0)
            ones_col = cpool.tile([128, 1], f32r)
            nc.vector.tensor_copy(ones_col[:], ones_col_f[:])
            ones_row_f = cpool.tile([1, 128], f32)
            nc.vector.memset(ones_row_f[:], 1.0)
            ones_row = cpool.tile([1, 128], f32r)
            nc.vector.tensor_copy(ones_row[:], ones_row_f[:])
            ones8 = cpool.tile([128, 8], f32)
            nc.vector.memset(ones8[:], 1.0)
            eps1 = cpool.tile([1, 1], f32)
            nc.vector.memset(eps1[:], EPS)
            post_t = cpool.tile([128, 4, N], f32)
            nc.sync.dma_start(post_t[:], post_d[:])
            skemo_t = cpool.tile([128, 4, BC], f32)
            nc.sync.dma_start(skemo_t[:], skemo_d[:])

            h_t = hpool.tile([128, 4, BC * N], f32r)

            # ---- embedding ------------------------------------------------
            for i in range(BC * 4):  # b = i // 4, tt = i % 4
                b, tt = divmod(i, 4)
                idx_t = upool.tile([128, 1], i32, tag="idx")
                nc.sync.dma_start(idx_t[:], xids_d[i].rearrange("(p o) -> p o", o=1))
                g_t = upool.tile([128, D], f32, tag="u")
                nc.gpsimd.indirect_dma_start(
                    out=g_t[:], out_offset=None, in_=tok_d[:],
                    in_offset=bass.IndirectOffsetOnAxis(ap=idx_t[:, :1], axis=0),
                )
                for c in range(4):
                    tp = ps_mm.tile([128, 512], f32, tag="mm")
                    nc.tensor.transpose(tp[:, :128], g_t[:, c * 128:(c + 1) * 128], ident[:])
                    dst = h_t[:, c, i * 128:(i + 1) * 128]
                    nc.vector.tensor_add(dst, tp[:, :128],
                                         post_t[:, c, tt * 128:(tt + 1) * 128])
                    nc.vector.tensor_scalar(
                        dst, dst, skemo_t[:, c, b:b + 1], None,
                        mybir.AluOpType.add)

            # ---- helpers --------------------------------------------------
            def layernorm(bsl, lnw, lnb):
                """LN over features of h[:, :, bsl] -> fp32r tile [128,4,N]."""
                a_t = big.tile([128, 4, N], f32r, tag="big")
                stat_s = ps_st.tile([1, 512], f32, tag="st")
                stat_q = ps_st.tile([1, 512], f32, tag="st")
                for c in range(4):
                    sq = upool.tile([128, N], f32r, tag="u")
                    nc.vector.tensor_mul(sq[:], h_t[:, c, bsl], h_t[:, c, bsl])
                    nc.tensor.matmul(stat_s[:], ones_col[:], h_t[:, c, bsl],
                                     start=(c == 0), stop=(c == 3))
                    nc.tensor.matmul(stat_q[:], ones_col[:], sq[:],
                                     start=(c == 0), stop=(c == 3))
                rows = rpool.tile([1, 3584], f32, tag="rows")
                # col blocks: 0=negmean 1=E[x^2] 2=mean^2 3=var 4=sqrt 5=r 6=c
                NM, EX, M2, VA, SQ, RR, CC = (slice(i * 512, (i + 1) * 512)
                                              for i in range(7))
                nc.vector.tensor_scalar(rows[:, NM], stat_s[:], -1.0 / D, None,
                                        mybir.AluOpType.mult)
                nc.vector.tensor_scalar(rows[:, EX], stat_q[:], 1.0 / D, None,
                                        mybir.AluOpType.mult)
                nc.vector.tensor_mul(rows[:, M2], rows[:, NM], rows[:, NM])
                nc.vector.tensor_sub(rows[:, VA], rows[:, EX], rows[:, M2])
                nc.scalar.activation(rows[:, SQ], rows[:, VA], AF.Sqrt, bias=eps1[:, :])
                nc.vector.reciprocal(rows[:, RR], rows[:, SQ])
                nc.vector.tensor_mul(rows[:, CC], rows[:, NM], rows[:, RR])
                rowr = rpool.tile([1, 1024], f32r, tag="rowr")
                nc.vector.tensor_copy(rowr[:, 0:512], rows[:, RR])
                nc.vector.tensor_copy(rowr[:, 512:1024], rows[:, CC])
                rb = ps_st.tile([128, 512], f32, tag="st")
                nc.tensor.matmul(rb[:], ones_row[:], rowr[:, 0:512], start=True, stop=True)
                cb = ps_st.tile([128, 512], f32, tag="st")
                nc.tensor.matmul(cb[:], ones_row[:], rowr[:, 512:1024], start=True, stop=True)
                for c in range(4):
                    u_t = upool.tile([128, N], f32, tag="u")
                    nc.vector.tensor_mul(u_t[:], h_t[:, c, bsl], rb[:])
                    nc.vector.tensor_add(u_t[:], u_t[:], cb[:])
                    nc.vector.tensor_scalar(a_t[:, c, :], u_t[:],
                                            lnw[:, c:c + 1], lnb[:, c:c + 1],
                                            mybir.AluOpType.mult,
                                            mybir.AluOpType.add)
                return a_t

            def add_residual(ps, bias_col, bsl, c2):
                u_t = upool.tile([128, N], f32, tag="u")
                nc.vector.tensor_scalar(u_t[:], ps[:], bias_col, None,
                                        mybir.AluOpType.add)
                dst = h_t[:, c2, bsl]
                nc.vector.tensor_add(dst, dst, u_t[:])

            # ---- transformer layers --------------------------------------
            for L in range(depth):
                wqk_t = wpool.tile([128, 4, 1024], f32r, tag="wqk")
                nc.sync.dma_start(wqk_t[:], wqk_d[L])
                wv_t = wpool.tile([128, 4, 512], f32r, tag="wv")
                nc.sync.dma_start(wv_t[:], wv_d[L])
                wproj_t = wpool.tile([128, 4, 512], f32r, tag="wproj")
                nc.sync.dma_start(wproj_t[:], wproj_d[L])
                qkb_t = bpool.tile([128, 8], f32, tag="qkb")
                nc.sync.dma_start(qkb_t[:], qkb_d[L])
                vb_t = bpool.tile([128, 4], f32, tag="vb")
                nc.sync.dma_start(vb_t[:], vb_d[L])
                projb_t = bpool.tile([128, 4], f32, tag="projb")
                nc.sync.dma_start(projb_t[:], projb_d[L])
                ln1w_t = bpool.tile([128, 4], f32, tag="ln1w")
                nc.sync.dma_start(ln1w_t[:], ln1w_d[L])
                ln1b_t = bpool.tile([128, 4], f32, tag="ln1b")
                nc.sync.dma_start(ln1b_t[:], ln1b_d[L])

                # ===== attention =====
                for b in range(BC):
                    bsl = slice(b * N, (b + 1) * N)
                    a_t = layernorm(bsl, ln1w_t, ln1b_t)
                    q_t = big.tile([128, 4, N], f32r, tag="big")
                    k_t = big.tile([128, 4, N], f32r, tag="big")
                    for ob in range(8):
                        ps = ps_mm.tile([128, 512], f32, tag="mm")
                        for c in range(4):
                            nc.tensor.matmul(ps[:], wqk_t[:, c, ob * 128:(ob + 1) * 128],
                                             a_t[:, c, :], start=(c == 0), stop=(c == 3))
                        dst = q_t[:, ob, :] if ob < 4 else k_t[:, ob - 4, :]
                        nc.scalar.activation(dst, ps[:], AF.Identity,
                                             bias=qkb_t[:, ob:ob + 1])
                    v_t = big.tile([128, 4, H * 65], f32r, tag="big")
                    for tb in range(4):
                        ps = ps_mm.tile([128, 512], f32, tag="mm")
                        for c in range(4):
                            nc.tensor.matmul(ps[:], a_t[:, c, tb * 128:(tb + 1) * 128],
                                             wv_t[:, c, :], start=(c == 0), stop=(c == 3))
                        vv = v_t[:, tb, :].rearrange("p (h e) -> p h e", e=65)
                        nc.vector.tensor_copy(
                            vv[:, :, 0:64],
                            ps[:].rearrange("p (h e) -> p h e", e=64))
                        nc.vector.tensor_copy(
                            vv[:, :, 64:65],
                            ones8[:].rearrange("p (h o) -> p h o", o=1))
                    oT = big.tile([128, 4, N], f32r, tag="big")
                    for hh in range(H):
                        part = (hh % 2) * 64
                        ch = hh // 2
                        q_h = q_t[part:part + 64, ch, :]
                        k_h = k_t[part:part + 64, ch, :]
                        exps = []
                        for kc in range(4):
                            sc = ps_sc.tile([128, 512], f32, tag="sc")
                            nc.tensor.matmul(sc[:], k_h[:, kc * 128:(kc + 1) * 128],
                                             q_h, start=True, stop=True)
                            e_t = expp.tile([128, N], f32r, tag="exp")
                            nc.scalar.activation(e_t[:], sc[:], AF.Exp,
                                                 scale=float(HD) ** -0.5)
                            exps.append(e_t)
                        o_ps = ps_o.tile([65, 512], f32, tag="o")
                        for kc in range(4):
                            nc.tensor.matmul(o_ps[:],
                                             v_t[:, kc, hh * 65:(hh + 1) * 65],
                                             exps[kc][:], start=(kc == 0), stop=(kc == 3))
                        recf = rpool.tile([1, 512], f32, tag="recf")
                        nc.vector.reciprocal(recf[:], o_ps[64:65, :])
                        recr = rpool.tile([1, 512], f32r, tag="recr")
                        nc.vector.tensor_copy(recr[:], recf[:])
                        rb = ps_st.tile([128, 512], f32, tag="st")
                        nc.tensor.matmul(rb[:], ones_row[:], recr[:],
                                         start=True, stop=True)
                        oev = upool.tile([64, 512], f32, tag="u")
                        nc.scalar.activation(oev[:], o_ps[0:64, :], AF.Copy)
                        dst = oT[part:part + 64, ch, :]
                        nc.vector.tensor_mul(dst, oev[:], rb[0:64, :])
                        nc.vector.tensor_scalar(dst, dst, vb_t[part:part + 64, ch:ch + 1],
                                                None, mybir.AluOpType.add)
                    for c2 in range(4):
                        ps = ps_mm.tile([128, 512], f32, tag="mm")
                        for c in range(4):
                            nc.tensor.matmul(ps[:], wproj_t[:, c, c2 * 128:(c2 + 1) * 128],
                                             oT[:, c, :], start=(c == 0), stop=(c == 3))
                        add_residual(ps, projb_t[:, c2:c2 + 1], bsl, c2)

                # ===== MLP =====
                fc1b_t = bpool.tile([128, 16], f32, tag="fc1b")
                nc.sync.dma_start(fc1b_t[:], fc1b_d[L])
                fc2b_t = bpool.tile([128, 4], f32, tag="fc2b")
                nc.sync.dma_start(fc2b_t[:], fc2b_d[L])
                ln2w_t = bpool.tile([128, 4], f32, tag="ln2w")
                nc.sync.dma_start(ln2w_t[:], ln2w_d[L])
                ln2b_t = bpool.tile([128, 4], f32, tag="ln2b")
                nc.sync.dma_start(ln2b_t[:], ln2b_d[L])

                for b in range(BC):
                    bsl = slice(b * N, (b + 1) * N)
                    a2_t = layernorm(bsl, ln2w_t, ln2b_t)
                    m_ts = [big.tile([128, 4, N], f32r, tag="big", name=f"m{i}") for i in range(4)]
                    for fbp in range(8):
                        wfc1_t = wbpool.tile([128, 4, 256], f32r, tag="wfc1b")
                        nc.sync.dma_start(wfc1_t[:], wfc1_d[L][:, :, fbp * 256:(fbp + 1) * 256])
                        for sub in range(2):
                            fb = fbp * 2 + sub
                            ps = ps_mm.tile([128, 512], f32, tag="mm")
                            for c in range(4):
                                nc.tensor.matmul(ps[:], wfc1_t[:, c, sub * 128:(sub + 1) * 128],
                                                 a2_t[:, c, :], start=(c == 0), stop=(c == 3))
                            nc.scalar.activation(m_ts[fb // 4][:, fb % 4, :], ps[:],
                                                 AF.Gelu, bias=fc1b_t[:, fb:fb + 1])
                    for c2 in range(4):
                        wfc2_t = wbpool.tile([128, 16, 128], f32r, tag="wfc2b")
                        nc.sync.dma_start(wfc2_t[:], wfc2_d[L][:, :, c2 * 128:(c2 + 1) * 128])
                        ps = ps_mm.tile([128, 512], f32, tag="mm")
                        for fb in range(16):
                            nc.tensor.matmul(ps[:], wfc2_t[:, fb, :],
                                             m_ts[fb // 4][:, fb % 4, :],
                                             start=(fb == 0), stop=(fb == 15))
                        add_residual(ps, fc2b_t[:, c2:c2 + 1], bsl, c2)

            # ---- final norm + pooling ------------------------------------
            normw_t = bpool.tile([128, 4], f32, tag="normw")
            nc.sync.dma_start(normw_t[:], normw_d[:])
            normb_t = bpool.tile([128, 4], f32, tag="normb")
            nc.sync.dma_start(normb_t[:], normb_d[:])
            pooledT = cpool.tile([128, 4, BC], f32r)
            for b in range(BC):
                bsl = slice(b * N, (b + 1) * N)
                hln = layernorm(bsl, normw_t, normb_t)
                with nc.allow_low_precision(reason="fp32r pooled rounding"):
                    for c in range(4):
                        nc.vector.tensor_reduce(
                            out=pooledT[:, c, b:b + 1], in_=hln[:, c, :],
                            op=mybir.AluOpType.add, axis=mybir.AxisListType.X)
            nc.vector.tensor_scalar(pooledT[:], pooledT[:], 1.0 / N, None,
                                    mybir.AluOpType.mult)
            for c in range(4):
                nc.sync.dma_start(
                    pooled_d[:, c * 128:(c + 1) * 128].rearrange("b p -> p b"),
                    pooledT[:, c, :].bitcast(f32))

            # ---- heads ----------------------------------------------------
            wslot = {1024: "wqk", 512: "wv"}
            for nm, O1, act1, O2 in HEADS:
                w1d, b1d, w2d, b2td = head_dram[nm]
                nb1 = O1 // 128
                w1_t = wpool.tile([128, 4, O1], f32r, tag=wslot[O1])
                nc.sync.dma_start(w1_t[:], w1d[:])
                O2p = O2 + (O2 % 2)
                w2_t = wpool.tile([128, nb1, O2p], f32r,
                                  tag=("wproj" if nb1 * O2p <= 4 * 512 else "wqk"))
                nc.sync.dma_start(w2_t[:], w2d[:])
                b1_t = bpool.tile([128, nb1], f32, tag=f"{nm}1b")
                nc.sync.dma_start(b1_t[:], b1d[:])
                b2_t = bpool.tile([BC, O2], f32, tag=f"{nm}2bt")
                nc.sync.dma_start(b2_t[:], b2td[:])
                t1 = big.tile([128, nb1, BC], f32r, tag="big")
                for ob in range(nb1):
                    ps = ps_mm.tile([128, 512], f32, tag="mm")
                    for c in range(4):
                        nc.tensor.matmul(ps[:, 0:BC], w1_t[:, c, ob * 128:(ob + 1) * 128],
                                         pooledT[:, c, :], start=(c == 0), stop=(c == 3))
                    nc.scalar.activation(t1[:, ob, :], ps[:, 0:BC], act1,
                                         bias=b1_t[:, ob:ob + 1])
                ps2 = ps_mm.tile([128, 512], f32, tag="mm")
                for j2 in range(nb1):
                    nc.tensor.matmul(ps2[0:BC, 0:O2p], t1[:, j2, :], w2_t[:, j2, :],
                                     start=(j2 == 0), stop=(j2 == nb1 - 1))
                hout = upool.tile([BC, 512], f32, tag="u")
                nc.vector.tensor_add(hout[:, 0:O2], ps2[0:BC, 0:O2], b2_t[:])
                if nm in ("sk", "co"):
                    nc.scalar.activation(hout[:, 0:O2], hout[:, 0:O2], AF.Sigmoid)
                elif nm == "em":
                    mx = rpool.tile([BC, 2], f32, tag="mx")
                    nc.vector.tensor_reduce(out=mx[:, 0:1], in_=hout[:, 0:O2],
                                            op=mybir.AluOpType.max,
                                            axis=mybir.AxisListType.X)
                    nc.vector.tensor_scalar(mx[:, 1:2], mx[:, 0:1], -1.0, None,
                                            mybir.AluOpType.mult)
                    sm = rpool.tile([BC, 2], f32, tag="sm")
                    nc.scalar.activation(hout[:, 0:O2], hout[:, 0:O2], AF.Exp,
                                         bias=mx[:, 1:2], accum_out=sm[:, 0:1])
                    nc.vector.reciprocal(sm[:, 1:2], sm[:, 0:1])
                    nc.vector.tensor_scalar(hout[:, 0:O2], hout[:, 0:O2],
                                            sm[:, 1:2], None, mybir.AluOpType.mult)
                nc.sync.dma_start(out_head_d[nm][:], hout[:, 0:O2])

    nc.compile()
    return nc


# ----------------------------------------------------------------------------
# host side
# ----------------------------------------------------------------------------
def _prep_inputs(x, skill_ids, emotion_ids, p, depth):
    """Build the per-core input maps."""
    x = np.asarray(x).astype(np.int32)
    skill_ids = np.asarray(skill_ids).astype(np.int64)
    emotion_ids = np.asarray(emotion_ids).astype(np.int64)

    def f32(a):
        return np.ascontiguousarray(np.asarray(a), dtype=np.float32)

    blocks = p["blocks"]
    qkv_w = f32(blocks["qkv_w"])      # [depth, 1536, 512]
    qkv_b = f32(blocks["qkv_b"])      # [depth, 1536]
    proj_w = f32(blocks["proj_w"])    # [depth, 512, 512]
    proj_b = f32(blocks["proj_b"])
    fc1_w = f32(blocks["fc1_w"])      # [depth, 2048, 512]
    fc1_b = f32(blocks["fc1_b"])
    fc2_w = f32(blocks["fc2_w"])      # [depth, 512, 2048]
    fc2_b = f32(blocks["fc2_b"])

    def chunkT(w):
        # [depth, O, Din] -> [depth, 128, Din//128, O]  (feature-major lhsT)
        d, O, Di = w.shape
        return np.ascontiguousarray(
            w.transpose(0, 2, 1).reshape(d, Di // 128, 128, O).transpose(0, 2, 1, 3))

    def colchunk(v):
        # [depth, Dout] -> [depth, 128, Dout//128]
        d, O = v.shape
        return np.ascontiguousarray(v.reshape(d, O // 128, 128).transpose(0, 2, 1))

    shared = {
        "tok": f32(p["tok"]),
        "posT": np.ascontiguousarray(
            f32(p["pos"]).T.reshape(4, 128, N).transpose(1, 0, 2)),
        "wqk": chunkT(qkv_w[:, :1024, :]),
        "wv": chunkT(qkv_w[:, 1024:, :]),
        "wproj": chunkT(proj_w),
        "wfc1": chunkT(fc1_w),
        "wfc2": chunkT(fc2_w),
        "qkb": colchunk(qkv_b[:, :1024]),
        "vb": colchunk(qkv_b[:, 1024:]),
        "projb": colchunk(proj_b),
        "fc1b": colchunk(fc1_b),
        "fc2b": colchunk(fc2_b),
        "ln1w": colchunk(f32(blocks["ln1_w"])),
        "ln1b": colchunk(f32(blocks["ln1_b"])),
        "ln2w": colchunk(f32(blocks["ln2_w"])),
        "ln2b": colchunk(f32(blocks["ln2_b"])),
        "normw": np.ascontiguousarray(f32(p["norm_w"]).reshape(4, 128).T),
        "normb": np.ascontiguousarray(f32(p["norm_b"]).reshape(4, 128).T),
    }
    headmap = {
        "sk": ("sk1_w", "sk1_b", "sk2_w", "sk2_b"),
        "em": ("em1_w", "em1_b", "em2_w", "em2_b"),
        "co": ("co1_w", "co1_b", "co2_w", "co2_b"),
        "cr": ("cr1_w", "cr1_b", "cr2_w", "cr2_b"),
        "wi": ("wi1_w", "wi1_b", "wi2_w", "wi2_b"),
        "im": ("im1_w", "im1_b", "im2_w", "im2_b"),
    }
    for nm, (w1k, b1k, w2k, b2k) in headmap.items():
        w1 = f32(p[w1k])  # [O1, 512]
        w2 = f32(p[w2k])  # [O2, O1]
        b1 = f32(p[b1k])
        b2 = f32(p[b2k])
        O1 = w1.shape[0]
        O2 = w2.shape[0]
        shared[f"{nm}1w"] = np.ascontiguousarray(
            w1.T.reshape(4, 128, O1).transpose(1, 0, 2))
        shared[f"{nm}1b"] = np.ascontiguousarray(b1.reshape(O1 // 128, 128).T)
        O2p = O2 + (O2 % 2)
        w2p = np.zeros((O2p, O1), np.float32)
        w2p[:O2] = w2
        shared[f"{nm}2w"] = np.ascontiguousarray(
            w2p.T.reshape(O1 // 128, 128, O2p).transpose(1, 0, 2))
        shared[f"{nm}2bt"] = np.ascontiguousarray(
            np.broadcast_to(b2[None, :], (BC, O2)))

    skill = f32(p["skill"])
    emo = f32(p["emo"])
    in_maps = []
    for core in range(NCORES):
        bs = slice(core * BC, (core + 1) * BC)
        skemo = skill[skill_ids[bs]] + emo[emotion_ids[bs]]  # [BC, 512]
        m = dict(shared)
        m["xids"] = np.ascontiguousarray(
            x[bs].reshape(BC * 4, 128))
        m["skemoT"] = np.ascontiguousarray(
            skemo.T.reshape(4, 128, BC).transpose(1, 0, 2))
        in_maps.append(m)
    return in_maps


def kernel(x, skill_ids, emotion_ids, params):
    global _PROG, LAST_RUN
    from concourse.bass_utils import run_bass_kernel_spmd

    if _PROG is None:
        _PROG = _build_program(DEPTH)
    nc = _PROG

    in_maps = _prep_inputs(x, skill_ids, emotion_ids, params, DEPTH)
    trace = os.environ.get("KERNEL_TRACE", "") == "1"
    res = run_bass_kernel_spmd(nc, in_maps, core_ids=list(range(NCORES)),
                               trace=trace)
    LAST_RUN = res

    pooled = np.concatenate([res.results[c]["pooled"] for c in range(NCORES)], 0)
    skills = np.concatenate([res.results[c]["skills"] for c in range(NCORES)], 0)
    emotions = np.concatenate([res.results[c]["emotions"] for c in range(NCORES)], 0)
    consc = np.concatenate([res.results[c]["consc"] for c in range(NCORES)], 0)
    creat = np.concatenate([res.results[c]["creat"] for c in range(NCORES)], 0)
    wisdom = np.concatenate([res.results[c]["wisdom"] for c in range(NCORES)], 0)
    improv = np.concatenate([res.results[c]["improv"] for c in range(NCORES)], 0)
    return (pooled, skills, emotions, consc, creat, wisdom, improv)


# revision 10
# speedup vs baseline: 5.8458x; 5.8458x over previous
"""ConsciousnessTransformer Trainium2 kernel.

Data-parallel over batch: 8 NeuronCores x 4 sequences each. Activations are
kept feature-major ([d, token]) in SBUF so every linear contracts over the
partition dim. Matmuls run in float32r (full-rate on PE, ~tf32 accuracy);
elementwise math stays fp32. LayerNorm stats and row->tile broadcasts use
ones-vector matmuls on the PE; attention softmax runs along the partition
axis with the normalizer folded in as a ones-column of V.
"""

import os
import sys

sys.path.insert(0, "/opt/trn_rl_repo")

import numpy as np

D = 512
DEPTH = 12
H = 8
HD = 64
V = 10000
B = 32
N = 512
FF = 2048
EPS = 1e-5
NCORES = 8
BC = B // NCORES  # sequences per core
NB = 50  # skills
NE = 25  # emotions

_PROG = None  # (nc, input_names)
LAST_RUN = None  # BassKernelResults of the most recent execution


# ----------------------------------------------------------------------------
# device program
# ----------------------------------------------------------------------------
def _build_program(depth):
    import concourse.bass as bass
    import concourse.tile as tile
    from concourse import bacc, mybir
    from concourse.masks import make_identity

    f32 = mybir.dt.float32
    f32r = mybir.dt.float32r
    i32 = mybir.dt.int32
    AF = mybir.ActivationFunctionType

    nc = bacc.Bacc("TRN2", target_bir_lowering=False, debug=False,
                   num_devices=NCORES)

    # ---- DRAM inputs -------------------------------------------------------
    xids_d = nc.dram_tensor("xids", [BC * 4, 128], i32, kind="ExternalInput")
    tok_d = nc.dram_tensor("tok", [V, D], f32, kind="ExternalInput")
    post_d = nc.dram_tensor("posT", [128, 4, N], f32, kind="ExternalInput")
    skemo_d = nc.dram_tensor("skemoT", [128, 4, BC], f32, kind="ExternalInput")

    wqk_d = nc.dram_tensor("wqk", [depth, 128, 4, 1024], f32r, kind="ExternalInput")
    wv_d = nc.dram_tensor("wv", [depth, 128, 4, 512], f32r, kind="ExternalInput")
    wproj_d = nc.dram_tensor("wproj", [depth, 128, 4, 512], f32r, kind="ExternalInput")
    wfc1_d = nc.dram_tensor("wfc1", [depth, 128, 4, FF], f32r, kind="ExternalInput")
    wfc2_d = nc.dram_tensor("wfc2", [depth, 128, 16, 512], f32r, kind="ExternalInput")

    qkb_d = nc.dram_tensor("qkb", [depth, 128, 8], f32, kind="ExternalInput")
    vb_d = nc.dram_tensor("vb", [depth, 128, 4], f32, kind="ExternalInput")
    projb_d = nc.dram_tensor("projb", [depth, 128, 4], f32, kind="ExternalInput")
    fc1b_d = nc.dram_tensor("fc1b", [depth, 128, 16], f32, kind="ExternalInput")
    fc2b_d = nc.dram_tensor("fc2b", [depth, 128, 4], f32, kind="ExternalInput")
    ln1w_d = nc.dram_tensor("ln1w", [depth, 128, 4], f32, kind="ExternalInput")
    ln1b_d = nc.dram_tensor("ln1b", [depth, 128, 4], f32, kind="ExternalInput")
    ln2w_d = nc.dram_tensor("ln2w", [depth, 128, 4], f32, kind="ExternalInput")
    ln2b_d = nc.dram_tensor("ln2b", [depth, 128, 4], f32, kind="ExternalInput")
    normw_d = nc.dram_tensor("normw", [128, 4], f32, kind="ExternalInput")
    normb_d = nc.dram_tensor("normb", [128, 4], f32, kind="ExternalInput")

    # heads: (name, O1, act1, O2)
    HEADS = [
        ("sk", 1024, AF.Gelu, NB),
        ("em", 512, AF.Gelu, NE),
        ("co", 512, AF.Gelu, 1),
        ("cr", 1024, AF.Gelu, 512),
        ("wi", 512, AF.Tanh, 256),
        ("im", 512, AF.Gelu, 512),
    ]
    head_dram = {}
    for nm, O1, _, O2 in HEADS:
        O2p = O2 + (O2 % 2)
        head_dram[nm] = (
            nc.dram_tensor(f"{nm}1w", [128, 4, O1], f32r, kind="ExternalInput"),
            nc.dram_tensor(f"{nm}1b", [128, O1 // 128], f32, kind="ExternalInput"),
            nc.dram_tensor(f"{nm}2w", [128, O1 // 128, O2p], f32r, kind="ExternalInput"),
            nc.dram_tensor(f"{nm}2bt", [BC, O2], f32, kind="ExternalInput"),
        )

    # ---- DRAM outputs ------------------------------------------------------
    pooled_d = nc.dram_tensor("pooled", [BC, D], f32, kind="ExternalOutput")
    out_head_d = {
        "sk": nc.dram_tensor("skills", [BC, NB], f32, kind="ExternalOutput"),
        "em": nc.dram_tensor("emotions", [BC, NE], f32, kind="ExternalOutput"),
        "co": nc.dram_tensor("consc", [BC, 1], f32, kind="ExternalOutput"),
        "cr": nc.dram_tensor("creat", [BC, 512], f32, kind="ExternalOutput"),
        "wi": nc.dram_tensor("wisdom", [BC, 256], f32, kind="ExternalOutput"),
        "im": nc.dram_tensor("improv", [BC, 512], f32, kind="ExternalOutput"),
    }

    with tile.TileContext(nc) as tc:
        with (
            tc.tile_pool(name="const", bufs=1) as cpool,
            tc.tile_pool(name="hres", bufs=1) as hpool,
            tc.tile_pool(name="w", bufs=1) as wpool,
            tc.tile_pool(name="wb", bufs=2) as wbpool,
            tc.tile_pool(name="bias", bufs=1) as bpool,
            tc.tile_pool(name="big", bufs=6) as big,
            tc.tile_pool(name="expp", bufs=8) as expp,
            tc.tile_pool(name="u", bufs=4) as upool,
            tc.tile_pool(name="rows", bufs=1) as rpool,
            tc.tile_pool(name="ps_mm", bufs=3, space="PSUM") as ps_mm,
            tc.tile_pool(name="ps_sc", bufs=2, space="PSUM") as ps_sc,
            tc.tile_pool(name="ps_o", bufs=1, space="PSUM") as ps_o,
            tc.tile_pool(name="ps_st", bufs=2, space="PSUM") as ps_st,
        ):
            # ---- constants -----------------------------------------------
            ident = cpool.tile([128, 128], f32)
            make_identity(nc, ident[:])
            ones_col_f = cpool.tile([128, 1], f32)
            nc.vector.memset(ones_col_f[:], 1.0)
            ones_col = cpool.tile([128, 1], f32r)
            nc.vector.tensor_copy(ones_col[:], ones_col_f[:])
            ones_row_f = cpool.tile([1, 128], f32)
            nc.vector.memset(ones_row_f[:], 1.0)
            ones_row = cpool.tile([1, 128], f32r)
            nc.vector.tensor_copy(ones_row[:], ones_row_f[:])
            ones8 = cpool.tile([128, 8], f32)
            nc.vector.memset(ones8[:], 1.0)
            ones_sq_f = cpool.tile([128, 128], f32)
            nc.vector.memset(ones_sq_f[:], 1.0)
            ones_sq = cpool.tile([128, 128], f32r)
            nc.vector.tensor_copy(ones_sq[:], ones_sq_f[:])
            eps1 = cpool.tile([1, 1], f32)
            nc.vector.memset(eps1[:], EPS)
            post_t = cpool.tile([128, 4, N], f32)
            nc.sync.dma_start(post_t[:], post_d[:])
            skemo_t = cpool.tile([128, 4, BC], f32)
            nc.sync.dma_start(skemo_t[:], skemo_d[:])

            h_t = hpool.tile([128, 4, BC * N], f32r)

            # ---- embedding ------------------------------------------------
            for i in range(BC * 4):  # b = i // 4, tt = i % 4
                b, tt = divmod(i, 4)
                idx_t = upool.tile([128, 1], i32, tag="idx")
                nc.sync.dma_start(idx_t[:], xids_d[i].rearrange("(p o) -> p o", o=1))
                g_t = upool.tile([128, D], f32, tag="u")
                nc.gpsimd.indirect_dma_start(
                    out=g_t[:], out_offset=None, in_=tok_d[:],
                    in_offset=bass.IndirectOffsetOnAxis(ap=idx_t[:, :1], axis=0),
                )
                for c in range(4):
                    tp = ps_mm.tile([128, 512], f32, tag="mm")
                    nc.tensor.transpose(tp[:, :128], g_t[:, c * 128:(c + 1) * 128], ident[:])
                    dst = h_t[:, c, i * 128:(i + 1) * 128]
                    nc.vector.tensor_add(dst, tp[:, :128],
                                         post_t[:, c, tt * 128:(tt + 1) * 128])
                    nc.vector.tensor_scalar(
                        dst, dst, skemo_t[:, c, b:b + 1], None,
                        mybir.AluOpType.add)

            # ---- helpers --------------------------------------------------
            def layernorm(bsl, lnw, lnb):
                """LN over features of h[:, :, bsl] -> fp32r tile [128,4,N]."""
                a_t = big.tile([128, 4, N], f32r, tag="big")
                stat_s = ps_st.tile([128, 512], f32, tag="st")
                stat_q = ps_st.tile([128, 512], f32, tag="st")
                for c in range(4):
                    sq = upool.tile([128, N], f32r, tag="u")
                    nc.vector.tensor_mul(sq[:], h_t[:, c, bsl], h_t[:, c, bsl])
                    nc.tensor.matmul(stat_s[:], ones_sq[:], h_t[:, c, bsl],
                                     start=(c == 0), stop=(c == 3))
                    nc.tensor.matmul(stat_q[:], ones_sq[:], sq[:],
                                     start=(c == 0), stop=(c == 3))
                rows = rpool.tile([1, 3584], f32, tag="rows")
                # col blocks: 0=negmean 1=E[x^2] 2=mean^2 3=var 4=sqrt 5=r 6=c
                NM, EX, M2, VA, SQ, RR, CC = (slice(i * 512, (i + 1) * 512)
                                              for i in range(7))
                nc.vector.tensor_scalar(rows[:, NM], stat_s[0:1, :], -1.0 / D, None,
                                        mybir.AluOpType.mult)
                nc.vector.tensor_scalar(rows[:, EX], stat_q[0:1, :], 1.0 / D, None,
                                        mybir.AluOpType.mult)
                nc.vector.tensor_mul(rows[:, M2], rows[:, NM], rows[:, NM])
                nc.vector.tensor_sub(rows[:, VA], rows[:, EX], rows[:, M2])
                nc.scalar.activation(rows[:, SQ], rows[:, VA], AF.Sqrt, bias=eps1[:, :])
                nc.vector.reciprocal_approx_fast(rows[:, RR], rows[:, SQ])
                nc.vector.tensor_mul(rows[:, CC], rows[:, NM], rows[:, RR])
                rowr = rpool.tile([1, 1024], f32r, tag="rowr")
                nc.vector.tensor_copy(rowr[:, 0:512], rows[:, RR])
                nc.vector.tensor_copy(rowr[:, 512:1024], rows[:, CC])
                rb = ps_st.tile([128, 512], f32, tag="st")
                nc.tensor.matmul(rb[:], ones_row[:], rowr[:, 0:512], start=True, stop=True)
                cb = ps_st.tile([128, 512], f32, tag="st")
                nc.tensor.matmul(cb[:], ones_row[:], rowr[:, 512:1024], start=True, stop=True)
                for c in range(4):
                    u_t = upool.tile([128, N], f32, tag="u")
                    nc.vector.tensor_mul(u_t[:], h_t[:, c, bsl], rb[:])
                    nc.vector.tensor_add(u_t[:], u_t[:], cb[:])
                    nc.vector.tensor_scalar(a_t[:, c, :], u_t[:],
                                            lnw[:, c:c + 1], lnb[:, c:c + 1],
                                            mybir.AluOpType.mult,
                                            mybir.AluOpType.add)
                return a_t

            def add_residual(ps, bias_col, bsl, c2):
                u_t = upool.tile([128, N], f32, tag="u")
                nc.vector.tensor_scalar(u_t[:], ps[:], bias_col, None,
                                        mybir.AluOpType.add)
                dst = h_t[:, c2, bsl]
                nc.vector.tensor_add(dst, dst, u_t[:])

            # ---- transformer layers --------------------------------------
            for L in range(depth):
                wqk_t = wpool.tile([128, 4, 1024], f32r, tag="wqk")
                nc.sync.dma_start(wqk_t[:], wqk_d[L])
                wv_t = wpool.tile([128, 4, 512], f32r, tag="wv")
                nc.sync.dma_start(wv_t[:], wv_d[L])
                wproj_t = wpool.tile([128, 4, 512], f32r, tag="wproj")
                nc.sync.dma_start(wproj_t[:], wproj_d[L])
                qkb_t = bpool.tile([128, 8], f32, tag="qkb")
                nc.sync.dma_start(qkb_t[:], qkb_d[L])
                vb_t = bpool.tile([128, 4], f32, tag="vb")
                nc.sync.dma_start(vb_t[:], vb_d[L])
                projb_t = bpool.tile([128, 4], f32, tag="projb")
                nc.sync.dma_start(projb_t[:], projb_d[L])
                ln1w_t = bpool.tile([128, 4], f32, tag="ln1w")
                nc.sync.dma_start(ln1w_t[:], ln1w_d[L])
                ln1b_t = bpool.tile([128, 4], f32, tag="ln1b")
                nc.sync.dma_start(ln1b_t[:], ln1b_d[L])

                # ===== attention =====
                for b in range(BC):
                    bsl = slice(b * N, (b + 1) * N)
                    a_t = layernorm(bsl, ln1w_t, ln1b_t)
                    q_t = big.tile([128, 4, N], f32r, tag="big")
                    k_t = big.tile([128, 4, N], f32r, tag="big")
                    for ob in range(8):
                        ps = ps_mm.tile([128, 512], f32, tag="mm")
                        for c in range(4):
                            nc.tensor.matmul(ps[:], wqk_t[:, c, ob * 128:(ob + 1) * 128],
                                             a_t[:, c, :], start=(c == 0), stop=(c == 3))
                        dst = q_t[:, ob, :] if ob < 4 else k_t[:, ob - 4, :]
                        nc.scalar.activation(dst, ps[:], AF.Identity,
                                             bias=qkb_t[:, ob:ob + 1])
                    v_t = big.tile([128, 4, H * 65], f32r, tag="big")
                    for tb in range(4):
                        ps = ps_mm.tile([128, 512], f32, tag="mm")
                        for c in range(4):
                            nc.tensor.matmul(ps[:], a_t[:, c, tb * 128:(tb + 1) * 128],
                                             wv_t[:, c, :], start=(c == 0), stop=(c == 3))
                        vv = v_t[:, tb, :].rearrange("p (h e) -> p h e", e=65)
                        nc.vector.tensor_copy(
                            vv[:, :, 0:64],
                            ps[:].rearrange("p (h e) -> p h e", e=64))
                        nc.vector.tensor_copy(
                            vv[:, :, 64:65],
                            ones8[:].rearrange("p (h o) -> p h o", o=1))
                    oT = big.tile([128, 4, N], f32r, tag="big")
                    for ch in range(4):
                        exps = {0: [], 1: []}
                        for kc in range(4):
                            for half in (0, 1):
                                part = half * 64
                                sc = ps_sc.tile([128, 512], f32, tag="sc")
                                nc.tensor.matmul(
                                    sc[:],
                                    k_t[part:part + 64, ch, kc * 128:(kc + 1) * 128],
                                    q_t[part:part + 64, ch, :],
                                    start=True, stop=True)
                                e_t = expp.tile([128, N], f32r, tag="exp")
                                nc.scalar.activation(e_t[:], sc[:], AF.Exp,
                                                     scale=float(HD) ** -0.5)
                                exps[half].append(e_t)
                        for half in (0, 1):
                            hh = 2 * ch + half
                            part = half * 64
                            o_ps = ps_o.tile([65, 512], f32, tag="o")
                            for kc in range(4):
                                nc.tensor.matmul(o_ps[:],
                                                 v_t[:, kc, hh * 65:(hh + 1) * 65],
                                                 exps[half][kc][:],
                                                 start=(kc == 0), stop=(kc == 3))
                            recf = rpool.tile([1, 512], f32, tag="recf")
                            nc.vector.reciprocal_approx_fast(recf[:], o_ps[64:65, :])
                            recr = rpool.tile([1, 512], f32r, tag="recr")
                            nc.vector.tensor_copy(recr[:], recf[:])
                            rb = ps_st.tile([128, 512], f32, tag="st")
                            nc.tensor.matmul(rb[:], ones_row[:], recr[:],
                                             start=True, stop=True)
                            oev = upool.tile([64, 512], f32, tag="u")
                            nc.scalar.activation(oev[:], o_ps[0:64, :], AF.Identity)
                            dst = oT[part:part + 64, ch, :]
                            nc.vector.tensor_mul(dst, oev[:], rb[0:64, :])
                            nc.vector.tensor_scalar(dst, dst,
                                                    vb_t[part:part + 64, ch:ch + 1],
                                                    None, mybir.AluOpType.add)
                    for c2 in range(4):
                        ps = ps_mm.tile([128, 512], f32, tag="mm")
                        for c in range(4):
                            nc.tensor.matmul(ps[:], wproj_t[:, c, c2 * 128:(c2 + 1) * 128],
                                             oT[:, c, :], start=(c == 0), stop=(c == 3))
                        add_residual(ps, projb_t[:, c2:c2 + 1], bsl, c2)

                # ===== MLP =====
                fc1b_t = bpool.tile([128, 16], f32, tag="fc1b")
                nc.sync.dma_start(fc1b_t[:], fc1b_d[L])
                fc2b_t = bpool.tile([128, 4], f32, tag="fc2b")
                nc.sync.dma_start(fc2b_t[:], fc2b_d[L])
                ln2w_t = bpool.tile([128, 4], f32, tag="ln2w")
                nc.sync.dma_start(ln2w_t[:], ln2w_d[L])
                ln2b_t = bpool.tile([128, 4], f32, tag="ln2b")
                nc.sync.dma_start(ln2b_t[:], ln2b_d[L])

                for b in range(BC):
                    bsl = slice(b * N, (b + 1) * N)
                    a2_t = layernorm(bsl, ln2w_t, ln2b_t)
                    m_ts = [big.tile([128, 4, N], f32r, tag="big", name=f"m{i}") for i in range(4)]
                    for fbp in range(8):
                        wfc1_t = wbpool.tile([128, 4, 256], f32r, tag="wfc1b")
                        nc.sync.dma_start(wfc1_t[:], wfc1_d[L][:, :, fbp * 256:(fbp + 1) * 256])
                        for sub in range(2):
                            fb = fbp * 2 + sub
                            ps = ps_mm.tile([128, 512], f32, tag="mm")
                            for c in range(4):
                                nc.tensor.matmul(ps[:], wfc1_t[:, c, sub * 128:(sub + 1) * 128],
                                                 a2_t[:, c, :], start=(c == 0), stop=(c == 3))
                            nc.scalar.activation(m_ts[fb // 4][:, fb % 4, :], ps[:],
                                                 AF.Gelu, bias=fc1b_t[:, fb:fb + 1])
                    for c2 in range(4):
                        wfc2_t = wbpool.tile([128, 16, 128], f32r, tag="wfc2b")
                        nc.sync.dma_start(wfc2_t[:], wfc2_d[L][:, :, c2 * 128:(c2 + 1) * 128])
                        ps = ps_mm.tile([128, 512], f32, tag="mm")
                        for fb in range(16):
                            nc.tensor.matmul(ps[:], wfc2_t[:, fb, :],
                                             m_ts[fb // 4][:, fb % 4, :],
                                             start=(fb == 0), stop=(fb == 15))
                        add_residual(ps, fc2b_t[:, c2:c2 + 1], bsl, c2)

            # ---- final norm + pooling ------------------------------------
            normw_t = bpool.tile([128, 4], f32, tag="normw")
            nc.sync.dma_start(normw_t[:], normw_d[:])
            normb_t = bpool.tile([128, 4], f32, tag="normb")
            nc.sync.dma_start(normb_t[:], normb_d[:])
            pooledT = cpool.tile([128, 4, BC], f32r)
            for b in range(BC):
                bsl = slice(b * N, (b + 1) * N)
                hln = layernorm(bsl, normw_t, normb_t)
                with nc.allow_low_precision(reason="fp32r pooled rounding"):
                    for c in range(4):
                        nc.vector.tensor_reduce(
                            out=pooledT[:, c, b:b + 1], in_=hln[:, c, :],
                            op=mybir.AluOpType.add, axis=mybir.AxisListType.X)
            nc.vector.tensor_scalar(pooledT[:], pooledT[:], 1.0 / N, None,
                                    mybir.AluOpType.mult)
            for c in range(4):
                nc.sync.dma_start(
                    pooled_d[:, c * 128:(c + 1) * 128].rearrange("b p -> p b"),
                    pooledT[:, c, :].bitcast(f32))

            # ---- heads ----------------------------------------------------
            wslot = {1024: "wqk", 512: "wv"}
            for nm, O1, act1, O2 in HEADS:
                w1d, b1d, w2d, b2td = head_dram[nm]
                nb1 = O1 // 128
                w1_t = wpool.tile([128, 4, O1], f32r, tag=wslot[O1])
                nc.sync.dma_start(w1_t[:], w1d[:])
                O2p = O2 + (O2 % 2)
                w2_t = wpool.tile([128, nb1, O2p], f32r,
                                  tag=("wproj" if nb1 * O2p <= 4 * 512 else "wqk"))
                nc.sync.dma_start(w2_t[:], w2d[:])
                b1_t = bpool.tile([128, nb1], f32, tag=f"{nm}1b")
                nc.sync.dma_start(b1_t[:], b1d[:])
                b2_t = bpool.tile([BC, O2], f32, tag=f"{nm}2bt")
                nc.sync.dma_start(b2_t[:], b2td[:])
                t1 = big.tile([128, nb1, BC], f32r, tag="big")
                for ob in range(nb1):
                    ps = ps_mm.tile([128, 512], f32, tag="mm")
                    for c in range(4):
                        nc.tensor.matmul(ps[:, 0:BC], w1_t[:, c, ob * 128:(ob + 1) * 128],
                                         pooledT[:, c, :], start=(c == 0), stop=(c == 3))
                    nc.scalar.activation(t1[:, ob, :], ps[:, 0:BC], act1,
                                         bias=b1_t[:, ob:ob + 1])
                ps2 = ps_mm.tile([128, 512], f32, tag="mm")
                for j2 in range(nb1):
                    nc.tensor.matmul(ps2[0:BC, 0:O2p], t1[:, j2, :], w2_t[:, j2, :],
                                     start=(j2 == 0), stop=(j2 == nb1 - 1))
                hout = upool.tile([BC, 512], f32, tag="u")
                nc.vector.tensor_add(hout[:, 0:O2], ps2[0:BC, 0:O2], b2_t[:])
                if nm in ("sk", "co"):
                    nc.scalar.activation(hout[:, 0:O2], hout[:, 0:O2], AF.Sigmoid)
                elif nm == "em":
                    mx = rpool.tile([BC, 2], f32, tag="mx")
                    nc.vector.tensor_reduce(out=mx[:, 0:1], in_=hout[:, 0:O2],
                                            op=mybir.AluOpType.max,
                                            axis=mybir.AxisListType.X)
                    nc.vector.tensor_scalar(mx[:, 1:2], mx[:, 0:1], -1.0, None,
                                            mybir.AluOpType.mult)
                    sm = rpool.tile([BC, 2], f32, tag="sm")
                    nc.scalar.activation(hout[:, 0:O2], hout[:, 0:O2], AF.Exp,
                                         bias=mx[:, 1:2], accum_out=sm[:, 0:1])
                    nc.vector.reciprocal_approx_fast(sm[:, 1:2], sm[:, 0:1])
                    nc.vector.tensor_scalar(hout[:, 0:O2], hout[:, 0:O2],
                                            sm[:, 1:2], None, mybir.AluOpType.mult)
                nc.sync.dma_start(out_head_d[nm][:], hout[:, 0:O2])

    nc.compile()
    return nc


# ----------------------------------------------------------------------------
# host side
# ----------------------------------------------------------------------------
def _prep_inputs(x, skill_ids, emotion_ids, p, depth):
    """Build the per-core input maps."""
    x = np.asarray(x).astype(np.int32)
    skill_ids = np.asarray(skill_ids).astype(np.int64)
    emotion_ids = np.asarray(emotion_ids).astype(np.int64)

    def f32(a):
        return np.ascontiguousarray(np.asarray(a), dtype=np.float32)

    blocks = p["blocks"]
    qkv_w = f32(blocks["qkv_w"])      # [depth, 1536, 512]
    qkv_b = f32(blocks["qkv_b"])      # [depth, 1536]
    proj_w = f32(blocks["proj_w"])    # [depth, 512, 512]
    proj_b = f32(blocks["proj_b"])
    fc1_w = f32(blocks["fc1_w"])      # [depth, 2048, 512]
    fc1_b = f32(blocks["fc1_b"])
    fc2_w = f32(blocks["fc2_w"])      # [depth, 512, 2048]
    fc2_b = f32(blocks["fc2_b"])

    def chunkT(w):
        # [depth, O, Din] -> [depth, 128, Din//128, O]  (feature-major lhsT)
        d, O, Di = w.shape
        return np.ascontiguousarray(
            w.transpose(0, 2, 1).reshape(d, Di // 128, 128, O).transpose(0, 2, 1, 3))

    def colchunk(v):
        # [depth, Dout] -> [depth, 128, Dout//128]
        d, O = v.shape
        return np.ascontiguousarray(v.reshape(d, O // 128, 128).transpose(0, 2, 1))

    shared = {
        "tok": f32(p["tok"]),
        "posT": np.ascontiguousarray(
            f32(p["pos"]).T.reshape(4, 128, N).transpose(1, 0, 2)),
        "wqk": chunkT(qkv_w[:, :1024, :]),
        "wv": chunkT(qkv_w[:, 1024:, :]),
        "wproj": chunkT(proj_w),
        "wfc1": chunkT(fc1_w),
        "wfc2": chunkT(fc2_w),
        "qkb": colchunk(qkv_b[:, :1024]),
        "vb": colchunk(qkv_b[:, 1024:]),
        "projb": colchunk(proj_b),
        "fc1b": colchunk(fc1_b),
        "fc2b": colchunk(fc2_b),
        "ln1w": colchunk(f32(blocks["ln1_w"])),
        "ln1b": colchunk(f32(blocks["ln1_b"])),
        "ln2w": colchunk(f32(blocks["ln2_w"])),
        "ln2b": colchunk(f32(blocks["ln2_b"])),
        "normw": np.ascontiguousarray(f32(p["norm_w"]).reshape(4, 128).T),
        "normb": np.ascontiguousarray(f32(p["norm_b"]).reshape(4, 128).T),
    }
    headmap = {
        "sk": ("sk1_w", "sk1_b", "sk2_w", "sk2_b"),
        "em": ("em1_w", "em1_b", "em2_w", "em2_b"),
        "co": ("co1_w", "co1_b", "co2_w", "co2_b"),
        "cr": ("cr1_w", "cr1_b", "cr2_w", "cr2_b"),
        "wi": ("wi1_w", "wi1_b", "wi2_w", "wi2_b"),
        "im": ("im1_w", "im1_b", "im2_w", "im2_b"),
    }
    for nm, (w1k, b1k, w2k, b2k) in headmap.items():
        w1 = f32(p[w1k])  # [O1, 512]
        w2 = f32(p[w2k])  # [O2, O1]
        b1 = f32(p[b1k])
        b2 = f32(p[b2k])
        O1 = w1.shape[0]
        O2 = w2.shape[0]
        shared[f"{nm}1w"] = np.ascontiguousarray(
            w1.T.reshape(4, 128, O1).transpose(1, 0, 2))
        shared[f"{nm}1b"] = np.ascontiguousarray(b1.reshape(O1 // 128, 128).T)
        O2p = O2 + (O2 % 2)
        w2p = np.zeros((O2p, O1), np.float32)
        w2p[:O2] = w2
        shared[f"{nm}2w"] = np.ascontiguousarray(
            w2p.T.reshape(O1 // 128, 128, O2p).transpose(1, 0, 2))
        shared[f"{nm}2bt"] = np.ascontiguousarray(
            np.broadcast_to(b2[None, :], (BC, O2)))

    skill = f32(p["skill"])
    emo = f32(p["emo"])
    in_maps = []
    for core in range(NCORES):
        bs = slice(core * BC, (core + 1) * BC)
        skemo = skill[skill_ids[bs]] + emo[emotion_ids[bs]]  # [BC, 512]
        m = dict(shared)
        m["xids"] = np.ascontiguousarray(
            x[bs].reshape(BC * 4, 128))
        m["skemoT"] = np.ascontiguousarray(
            skemo.T.reshape(4, 128, BC).transpose(1, 0, 2))
        in_maps.append(m)
    return in_maps


def kernel(x, skill_ids, emotion_ids, params):
    global _PROG, LAST_RUN
    from concourse.bass_utils import run_bass_kernel_spmd

    if _PROG is None:
        _PROG = _build_program(DEPTH)
    nc = _PROG

    in_maps = _prep_inputs(x, skill_ids, emotion_ids, params, DEPTH)
    trace = os.environ.get("KERNEL_TRACE", "") == "1"
    res = run_bass_kernel_spmd(nc, in_maps, core_ids=list(range(NCORES)),
                               trace=trace)
    LAST_RUN = res

    pooled = np.concatenate([res.results[c]["pooled"] for c in range(NCORES)], 0)
    skills = np.concatenate([res.results[c]["skills"] for c in range(NCORES)], 0)
    emotions = np.concatenate([res.results[c]["emotions"] for c in range(NCORES)], 0)
    consc = np.concatenate([res.results[c]["consc"] for c in range(NCORES)], 0)
    creat = np.concatenate([res.results[c]["creat"] for c in range(NCORES)], 0)
    wisdom = np.concatenate([res.results[c]["wisdom"] for c in range(NCORES)], 0)
    improv = np.concatenate([res.results[c]["improv"] for c in range(NCORES)], 0)
    return (pooled, skills, emotions, consc, creat, wisdom, improv)


# revision 11
# speedup vs baseline: 9.3406x; 1.5978x over previous
"""ConsciousnessTransformer Trainium2 kernel.

Data-parallel over batch: 8 NeuronCores x 4 sequences each. Activations are
kept feature-major ([d, token]) in SBUF so every linear contracts over the
partition dim. Matmuls run in float32r (full-rate on PE, ~tf32 accuracy);
elementwise math stays fp32. LayerNorm stats and row->tile broadcasts use
ones-vector matmuls on the PE; attention softmax runs along the partition
axis with the normalizer folded in as a ones-column of V.
"""

import os
import sys

sys.path.insert(0, "/opt/trn_rl_repo")

import numpy as np

D = 512
DEPTH = 12
H = 8
HD = 64
V = 10000
B = 32
N = 512
FF = 2048
EPS = 1e-5
NCORES = 8
BC = B // NCORES  # sequences per core
NB = 50  # skills
NE = 25  # emotions

_PROG = None  # (nc, input_names)
LAST_RUN = None  # BassKernelResults of the most recent execution


# ----------------------------------------------------------------------------
# device program
# ----------------------------------------------------------------------------
def _build_program(depth):
    import concourse.bass as bass
    import concourse.tile as tile
    from concourse import bacc, mybir
    from concourse.masks import make_identity

    f32 = mybir.dt.float32
    f32r = mybir.dt.float32r
    i32 = mybir.dt.int32
    AF = mybir.ActivationFunctionType

    nc = bacc.Bacc("TRN2", target_bir_lowering=False, debug=False,
                   num_devices=NCORES)

    # ---- DRAM inputs -------------------------------------------------------
    xids_d = nc.dram_tensor("xids", [BC * 4, 128], i32, kind="ExternalInput")
    tok_d = nc.dram_tensor("tok", [V, D], f32, kind="ExternalInput")
    post_d = nc.dram_tensor("posT", [128, 4, N], f32, kind="ExternalInput")
    skemo_d = nc.dram_tensor("skemoT", [128, 4, BC], f32, kind="ExternalInput")

    wqk_d = nc.dram_tensor("wqk", [depth, 128, 4, 1024], f32r, kind="ExternalInput")
    wv_d = nc.dram_tensor("wv", [depth, 128, 4, 512], f32r, kind="ExternalInput")
    wproj_d = nc.dram_tensor("wproj", [depth, 128, 4, 512], f32r, kind="ExternalInput")
    wfc1_d = nc.dram_tensor("wfc1", [depth, 128, 4, FF], f32r, kind="ExternalInput")
    wfc2_d = nc.dram_tensor("wfc2", [depth, 128, 16, 512], f32r, kind="ExternalInput")

    qkb_d = nc.dram_tensor("qkb", [depth, 128, 8], f32, kind="ExternalInput")
    vb_d = nc.dram_tensor("vb", [depth, 128, 4], f32, kind="ExternalInput")
    projb_d = nc.dram_tensor("projb", [depth, 128, 4], f32, kind="ExternalInput")
    fc1b_d = nc.dram_tensor("fc1b", [depth, 128, 16], f32, kind="ExternalInput")
    fc2b_d = nc.dram_tensor("fc2b", [depth, 128, 4], f32, kind="ExternalInput")
    ln1w_d = nc.dram_tensor("ln1w", [depth, 128, 4], f32, kind="ExternalInput")
    ln1b_d = nc.dram_tensor("ln1b", [depth, 128, 4], f32, kind="ExternalInput")
    ln2w_d = nc.dram_tensor("ln2w", [depth, 128, 4], f32, kind="ExternalInput")
    ln2b_d = nc.dram_tensor("ln2b", [depth, 128, 4], f32, kind="ExternalInput")
    normw_d = nc.dram_tensor("normw", [128, 4], f32, kind="ExternalInput")
    normb_d = nc.dram_tensor("normb", [128, 4], f32, kind="ExternalInput")

    # heads: (name, O1, act1, O2)
    HEADS = [
        ("sk", 1024, AF.Gelu, NB),
        ("em", 512, AF.Gelu, NE),
        ("co", 512, AF.Gelu, 1),
        ("cr", 1024, AF.Gelu, 512),
        ("wi", 512, AF.Tanh, 256),
        ("im", 512, AF.Gelu, 512),
    ]
    head_dram = {}
    for nm, O1, _, O2 in HEADS:
        O2p = O2 + (O2 % 2)
        head_dram[nm] = (
            nc.dram_tensor(f"{nm}1w", [128, 4, O1], f32r, kind="ExternalInput"),
            nc.dram_tensor(f"{nm}1b", [128, O1 // 128], f32, kind="ExternalInput"),
            nc.dram_tensor(f"{nm}2w", [128, O1 // 128, O2p], f32r, kind="ExternalInput"),
            nc.dram_tensor(f"{nm}2bt", [BC, O2], f32, kind="ExternalInput"),
        )

    # ---- DRAM outputs ------------------------------------------------------
    pooled_d = nc.dram_tensor("pooled", [BC, D], f32, kind="ExternalOutput")
    out_head_d = {
        "sk": nc.dram_tensor("skills", [BC, NB], f32, kind="ExternalOutput"),
        "em": nc.dram_tensor("emotions", [BC, NE], f32, kind="ExternalOutput"),
        "co": nc.dram_tensor("consc", [BC, 1], f32, kind="ExternalOutput"),
        "cr": nc.dram_tensor("creat", [BC, 512], f32, kind="ExternalOutput"),
        "wi": nc.dram_tensor("wisdom", [BC, 256], f32, kind="ExternalOutput"),
        "im": nc.dram_tensor("improv", [BC, 512], f32, kind="ExternalOutput"),
    }

    with tile.TileContext(nc) as tc:
        with (
            tc.tile_pool(name="const", bufs=1) as cpool,
            tc.tile_pool(name="hres", bufs=1) as hpool,
            tc.tile_pool(name="w", bufs=1) as wpool,
            tc.tile_pool(name="wb", bufs=2) as wbpool,
            tc.tile_pool(name="bias", bufs=1) as bpool,
            tc.tile_pool(name="big", bufs=6) as big,
            tc.tile_pool(name="expp", bufs=8) as expp,
            tc.tile_pool(name="u", bufs=4) as upool,
            tc.tile_pool(name="rows", bufs=1) as rpool,
            tc.tile_pool(name="ps_mm", bufs=3, space="PSUM") as ps_mm,
            tc.tile_pool(name="ps_sc", bufs=2, space="PSUM") as ps_sc,
            tc.tile_pool(name="ps_o", bufs=1, space="PSUM") as ps_o,
            tc.tile_pool(name="ps_st", bufs=2, space="PSUM") as ps_st,
        ):
            # ---- constants -----------------------------------------------
            ident = cpool.tile([128, 128], f32)
            make_identity(nc, ident[:])
            ones_col_f = cpool.tile([128, 1], f32)
            nc.vector.memset(ones_col_f[:], 1.0)
            ones_col = cpool.tile([128, 1], f32r)
            nc.vector.tensor_copy(ones_col[:], ones_col_f[:])
            ones_row_f = cpool.tile([1, 128], f32)
            nc.vector.memset(ones_row_f[:], 1.0)
            ones_row = cpool.tile([1, 128], f32r)
            nc.vector.tensor_copy(ones_row[:], ones_row_f[:])
            ones8 = cpool.tile([128, 8], f32)
            nc.vector.memset(ones8[:], 1.0)
            ones_sq_f = cpool.tile([128, 128], f32)
            nc.vector.memset(ones_sq_f[:], 1.0)
            ones_sq = cpool.tile([128, 128], f32r)
            nc.vector.tensor_copy(ones_sq[:], ones_sq_f[:])
            eps1 = cpool.tile([1, 1], f32)
            nc.vector.memset(eps1[:], EPS)
            post_t = cpool.tile([128, 4, N], f32)
            nc.sync.dma_start(post_t[:], post_d[:])
            skemo_t = cpool.tile([128, 4, BC], f32)
            nc.sync.dma_start(skemo_t[:], skemo_d[:])

            h_t = hpool.tile([128, 4, BC * N], f32r)

            # ---- embedding ------------------------------------------------
            for i in range(BC * 4):  # b = i // 4, tt = i % 4
                b, tt = divmod(i, 4)
                idx_t = upool.tile([128, 1], i32, tag="idx")
                nc.sync.dma_start(idx_t[:], xids_d[i].rearrange("(p o) -> p o", o=1))
                g_t = upool.tile([128, D], f32, tag="u")
                nc.gpsimd.indirect_dma_start(
                    out=g_t[:], out_offset=None, in_=tok_d[:],
                    in_offset=bass.IndirectOffsetOnAxis(ap=idx_t[:, :1], axis=0),
                )
                for c in range(4):
                    tp = ps_mm.tile([128, 512], f32, tag="mm")
                    nc.tensor.transpose(tp[:, :128], g_t[:, c * 128:(c + 1) * 128], ident[:])
                    dst = h_t[:, c, i * 128:(i + 1) * 128]
                    nc.vector.tensor_add(dst, tp[:, :128],
                                         post_t[:, c, tt * 128:(tt + 1) * 128])
                    nc.vector.tensor_scalar(
                        dst, dst, skemo_t[:, c, b:b + 1], None,
                        mybir.AluOpType.add)

            # ---- helpers --------------------------------------------------
            def layernorm(bsl, lnw, lnb):
                """LN over features of h[:, :, bsl] -> fp32r tile [128,4,N]."""
                a_t = big.tile([128, 4, N], f32r, tag="big")
                stat_s = ps_st.tile([128, 512], f32, tag="st")
                stat_q = ps_st.tile([128, 512], f32, tag="st")
                for c in range(4):
                    sq = upool.tile([128, N], f32r, tag="u")
                    nc.vector.tensor_mul(sq[:], h_t[:, c, bsl], h_t[:, c, bsl])
                    nc.tensor.matmul(stat_s[:], ones_sq[:], h_t[:, c, bsl],
                                     start=(c == 0), stop=(c == 3))
                    nc.tensor.matmul(stat_q[:], ones_sq[:], sq[:],
                                     start=(c == 0), stop=(c == 3))
                rows = rpool.tile([1, 3584], f32, tag="rows")
                # col blocks: 0=negmean 1=E[x^2] 2=mean^2 3=var 4=sqrt 5=r 6=c
                NM, EX, M2, VA, SQ, RR, CC = (slice(i * 512, (i + 1) * 512)
                                              for i in range(7))
                nc.vector.tensor_scalar(rows[:, NM], stat_s[0:1, :], -1.0 / D, None,
                                        mybir.AluOpType.mult)
                nc.vector.tensor_scalar(rows[:, EX], stat_q[0:1, :], 1.0 / D, None,
                                        mybir.AluOpType.mult)
                nc.vector.tensor_mul(rows[:, M2], rows[:, NM], rows[:, NM])
                nc.vector.tensor_sub(rows[:, VA], rows[:, EX], rows[:, M2])
                nc.scalar.activation(rows[:, SQ], rows[:, VA], AF.Sqrt, bias=eps1[:, :])
                nc.vector.reciprocal(rows[:, RR], rows[:, SQ])
                nc.vector.tensor_mul(rows[:, CC], rows[:, NM], rows[:, RR])
                rowr = rpool.tile([1, 1024], f32r, tag="rowr")
                nc.vector.tensor_copy(rowr[:, 0:512], rows[:, RR])
                nc.vector.tensor_copy(rowr[:, 512:1024], rows[:, CC])
                rb = ps_st.tile([128, 512], f32, tag="st")
                nc.tensor.matmul(rb[:], ones_row[:], rowr[:, 0:512], start=True, stop=True)
                cb = ps_st.tile([128, 512], f32, tag="st")
                nc.tensor.matmul(cb[:], ones_row[:], rowr[:, 512:1024], start=True, stop=True)
                for c in range(4):
                    u_t = upool.tile([128, N], f32, tag="u")
                    nc.vector.tensor_mul(u_t[:], h_t[:, c, bsl], rb[:])
                    nc.vector.tensor_add(u_t[:], u_t[:], cb[:])
                    nc.vector.tensor_scalar(a_t[:, c, :], u_t[:],
                                            lnw[:, c:c + 1], lnb[:, c:c + 1],
                                            mybir.AluOpType.mult,
                                            mybir.AluOpType.add)
                return a_t

            def add_residual(ps, bias_col, bsl, c2):
                u_t = upool.tile([128, N], f32, tag="u")
                nc.vector.tensor_scalar(u_t[:], ps[:], bias_col, None,
                                        mybir.AluOpType.add)
                dst = h_t[:, c2, bsl]
                nc.vector.tensor_add(dst, dst, u_t[:])

            # ---- transformer layers --------------------------------------
            for L in range(depth):
                wqk_t = wpool.tile([128, 4, 1024], f32r, tag="wqk")
                nc.sync.dma_start(wqk_t[:], wqk_d[L])
                wv_t = wpool.tile([128, 4, 512], f32r, tag="wv")
                nc.sync.dma_start(wv_t[:], wv_d[L])
                wproj_t = wpool.tile([128, 4, 512], f32r, tag="wproj")
                nc.sync.dma_start(wproj_t[:], wproj_d[L])
                qkb_t = bpool.tile([128, 8], f32, tag="qkb")
                nc.sync.dma_start(qkb_t[:], qkb_d[L])
                vb_t = bpool.tile([128, 4], f32, tag="vb")
                nc.sync.dma_start(vb_t[:], vb_d[L])
                projb_t = bpool.tile([128, 4], f32, tag="projb")
                nc.sync.dma_start(projb_t[:], projb_d[L])
                ln1w_t = bpool.tile([128, 4], f32, tag="ln1w")
                nc.sync.dma_start(ln1w_t[:], ln1w_d[L])
                ln1b_t = bpool.tile([128, 4], f32, tag="ln1b")
                nc.sync.dma_start(ln1b_t[:], ln1b_d[L])

                # ===== attention =====
                for b in range(BC):
                    bsl = slice(b * N, (b + 1) * N)
                    a_t = layernorm(bsl, ln1w_t, ln1b_t)
                    q_t = big.tile([128, 4, N], f32r, tag="big")
                    k_t = big.tile([128, 4, N], f32r, tag="big")
                    for ob in range(8):
                        ps = ps_mm.tile([128, 512], f32, tag="mm")
                        for c in range(4):
                            nc.tensor.matmul(ps[:], wqk_t[:, c, ob * 128:(ob + 1) * 128],
                                             a_t[:, c, :], start=(c == 0), stop=(c == 3))
                        dst = q_t[:, ob, :] if ob < 4 else k_t[:, ob - 4, :]
                        nc.scalar.activation(dst, ps[:], AF.Identity,
                                             bias=qkb_t[:, ob:ob + 1])
                    v_t = big.tile([128, 4, H * 65], f32r, tag="big")
                    for tb in range(4):
                        ps = ps_mm.tile([128, 512], f32, tag="mm")
                        for c in range(4):
                            nc.tensor.matmul(ps[:], a_t[:, c, tb * 128:(tb + 1) * 128],
                                             wv_t[:, c, :], start=(c == 0), stop=(c == 3))
                        vv = v_t[:, tb, :].rearrange("p (h e) -> p h e", e=65)
                        nc.vector.tensor_copy(
                            vv[:, :, 0:64],
                            ps[:].rearrange("p (h e) -> p h e", e=64))
                        nc.vector.tensor_copy(
                            vv[:, :, 64:65],
                            ones8[:].rearrange("p (h o) -> p h o", o=1))
                    oT = big.tile([128, 4, N], f32r, tag="big")
                    for ch in range(4):
                        exps = {0: [], 1: []}
                        for kc in range(4):
                            for half in (0, 1):
                                part = half * 64
                                sc = ps_sc.tile([128, 512], f32, tag="sc")
                                nc.tensor.matmul(
                                    sc[:],
                                    k_t[part:part + 64, ch, kc * 128:(kc + 1) * 128],
                                    q_t[part:part + 64, ch, :],
                                    start=True, stop=True)
                                e_t = expp.tile([128, N], f32r, tag="exp")
                                nc.scalar.activation(e_t[:], sc[:], AF.Exp,
                                                     scale=float(HD) ** -0.5)
                                exps[half].append(e_t)
                        for half in (0, 1):
                            hh = 2 * ch + half
                            part = half * 64
                            o_ps = ps_o.tile([65, 512], f32, tag="o")
                            for kc in range(4):
                                nc.tensor.matmul(o_ps[:],
                                                 v_t[:, kc, hh * 65:(hh + 1) * 65],
                                                 exps[half][kc][:],
                                                 start=(kc == 0), stop=(kc == 3))
                            recf = rpool.tile([1, 512], f32, tag="recf")
                            nc.vector.reciprocal(recf[:], o_ps[64:65, :])
                            recr = rpool.tile([1, 512], f32r, tag="recr")
                            nc.vector.tensor_copy(recr[:], recf[:])
                            rb = ps_st.tile([128, 512], f32, tag="st")
                            nc.tensor.matmul(rb[:], ones_row[:], recr[:],
                                             start=True, stop=True)
                            oev = upool.tile([64, 512], f32, tag="u")
                            nc.scalar.activation(oev[:], o_ps[0:64, :], AF.Identity)
                            dst = oT[part:part + 64, ch, :]
                            nc.vector.tensor_mul(dst, oev[:], rb[0:64, :])
                            nc.vector.tensor_scalar(dst, dst,
                                                    vb_t[part:part + 64, ch:ch + 1],
                                                    None, mybir.AluOpType.add)
                    for c2 in range(4):
                        ps = ps_mm.tile([128, 512], f32, tag="mm")
                        for c in range(4):
                            nc.tensor.matmul(ps[:], wproj_t[:, c, c2 * 128:(c2 + 1) * 128],
                                             oT[:, c, :], start=(c == 0), stop=(c == 3))
                        add_residual(ps, projb_t[:, c2:c2 + 1], bsl, c2)

                # ===== MLP =====
                fc1b_t = bpool.tile([128, 16], f32, tag="fc1b")
                nc.sync.dma_start(fc1b_t[:], fc1b_d[L])
                fc2b_t = bpool.tile([128, 4], f32, tag="fc2b")
                nc.sync.dma_start(fc2b_t[:], fc2b_d[L])
                ln2w_t = bpool.tile([128, 4], f32, tag="ln2w")
                nc.sync.dma_start(ln2w_t[:], ln2w_d[L])
                ln2b_t = bpool.tile([128, 4], f32, tag="ln2b")
                nc.sync.dma_start(ln2b_t[:], ln2b_d[L])

                for b in range(BC):
                    bsl = slice(b * N, (b + 1) * N)
                    a2_t = layernorm(bsl, ln2w_t, ln2b_t)
                    m_ts = [big.tile([128, 4, N], f32r, tag="big", name=f"m{i}") for i in range(4)]
                    for fbp in range(8):
                        wfc1_t = wbpool.tile([128, 4, 256], f32r, tag="wfc1b")
                        nc.sync.dma_start(wfc1_t[:], wfc1_d[L][:, :, fbp * 256:(fbp + 1) * 256])
                        for sub in range(2):
                            fb = fbp * 2 + sub
                            ps = ps_mm.tile([128, 512], f32, tag="mm")
                            for c in range(4):
                                nc.tensor.matmul(ps[:], wfc1_t[:, c, sub * 128:(sub + 1) * 128],
                                                 a2_t[:, c, :], start=(c == 0), stop=(c == 3))
                            nc.scalar.activation(m_ts[fb // 4][:, fb % 4, :], ps[:],
                                                 AF.Gelu, bias=fc1b_t[:, fb:fb + 1])
                    for c2 in range(4):
                        wfc2_t = wbpool.tile([128, 16, 128], f32r, tag="wfc2b")
                        nc.sync.dma_start(wfc2_t[:], wfc2_d[L][:, :, c2 * 128:(c2 + 1) * 128])
                        ps = ps_mm.tile([128, 512], f32, tag="mm")
                        for fb in range(16):
                            nc.tensor.matmul(ps[:], wfc2_t[:, fb, :],
                                             m_ts[fb // 4][:, fb % 4, :],
                                             start=(fb == 0), stop=(fb == 15))
                        add_residual(ps, fc2b_t[:, c2:c2 + 1], bsl, c2)

            # ---- final norm + pooling ------------------------------------
            normw_t = bpool.tile([128, 4], f32, tag="normw")
            nc.sync.dma_start(normw_t[:], normw_d[:])
            normb_t = bpool.tile([128, 4], f32, tag="normb")
            nc.sync.dma_start(normb_t[:], normb_d[:])
            pooledT = cpool.tile([128, 4, BC], f32r)
            for b in range(BC):
                bsl = slice(b * N, (b + 1) * N)
                hln = layernorm(bsl, normw_t, normb_t)
                with nc.allow_low_precision(reason="fp32r pooled rounding"):
                    for c in range(4):
                        nc.vector.tensor_reduce(
                            out=pooledT[:, c, b:b + 1], in_=hln[:, c, :],
                            op=mybir.AluOpType.add, axis=mybir.AxisListType.X)
            nc.vector.tensor_scalar(pooledT[:], pooledT[:], 1.0 / N, None,
                                    mybir.AluOpType.mult)
            for c in range(4):
                nc.sync.dma_start(
                    pooled_d[:, c * 128:(c + 1) * 128].rearrange("b p -> p b"),
                    pooledT[:, c, :].bitcast(f32))

            # ---- heads ----------------------------------------------------
            wslot = {1024: "wqk", 512: "wv"}
            for nm, O1, act1, O2 in HEADS:
                w1d, b1d, w2d, b2td = head_dram[nm]
                nb1 = O1 // 128
                w1_t = wpool.tile([128, 4, O1], f32r, tag=wslot[O1])
                nc.sync.dma_start(w1_t[:], w1d[:])
                O2p = O2 + (O2 % 2)
                w2_t = wpool.tile([128, nb1, O2p], f32r,
                                  tag=("wproj" if nb1 * O2p <= 4 * 512 else "wqk"))
                nc.sync.dma_start(w2_t[:], w2d[:])
                b1_t = bpool.tile([128, nb1], f32, tag=f"{nm}1b")
                nc.sync.dma_start(b1_t[:], b1d[:])
                b2_t = bpool.tile([BC, O2], f32, tag=f"{nm}2bt")
                nc.sync.dma_start(b2_t[:], b2td[:])
                t1 = big.tile([128, nb1, BC], f32r, tag="big")
                for ob in range(nb1):
                    ps = ps_mm.tile([128, 512], f32, tag="mm")
                    for c in range(4):
                        nc.tensor.matmul(ps[:, 0:BC], w1_t[:, c, ob * 128:(ob + 1) * 128],
                                         pooledT[:, c, :], start=(c == 0), stop=(c == 3))
                    nc.scalar.activation(t1[:, ob, :], ps[:, 0:BC], act1,
                                         bias=b1_t[:, ob:ob + 1])
                ps2 = ps_mm.tile([128, 512], f32, tag="mm")
                for j2 in range(nb1):
                    nc.tensor.matmul(ps2[0:BC, 0:O2p], t1[:, j2, :], w2_t[:, j2, :],
                                     start=(j2 == 0), stop=(j2 == nb1 - 1))
                hout = upool.tile([BC, 512], f32, tag="u")
                nc.vector.tensor_add(hout[:, 0:O2], ps2[0:BC, 0:O2], b2_t[:])
                if nm in ("sk", "co"):
                    nc.scalar.activation(hout[:, 0:O2], hout[:, 0:O2], AF.Sigmoid)
                elif nm == "em":
                    mx = rpool.tile([BC, 2], f32, tag="mx")
                    nc.vector.tensor_reduce(out=mx[:, 0:1], in_=hout[:, 0:O2],
                                            op=mybir.AluOpType.max,
                                            axis=mybir.AxisListType.X)
                    nc.vector.tensor_scalar(mx[:, 1:2], mx[:, 0:1], -1.0, None,
                                            mybir.AluOpType.mult)
                    sm = rpool.tile([BC, 2], f32, tag="sm")
                    nc.scalar.activation(hout[:, 0:O2], hout[:, 0:O2], AF.Exp,
                                         bias=mx[:, 1:2], accum_out=sm[:, 0:1])
                    nc.vector.reciprocal(sm[:, 1:2], sm[:, 0:1])
                    nc.vector.tensor_scalar(hout[:, 0:O2], hout[:, 0:O2],
                                            sm[:, 1:2], None, mybir.AluOpType.mult)
                nc.sync.dma_start(out_head_d[nm][:], hout[:, 0:O2])

    nc.compile()
    return nc


# ----------------------------------------------------------------------------
# host side
# ----------------------------------------------------------------------------
def _prep_inputs(x, skill_ids, emotion_ids, p, depth):
    """Build the per-core input maps."""
    x = np.asarray(x).astype(np.int32)
    skill_ids = np.asarray(skill_ids).astype(np.int64)
    emotion_ids = np.asarray(emotion_ids).astype(np.int64)

    def f32(a):
        return np.ascontiguousarray(np.asarray(a), dtype=np.float32)

    blocks = p["blocks"]
    qkv_w = f32(blocks["qkv_w"])      # [depth, 1536, 512]
    qkv_b = f32(blocks["qkv_b"])      # [depth, 1536]
    proj_w = f32(blocks["proj_w"])    # [depth, 512, 512]
    proj_b = f32(blocks["proj_b"])
    fc1_w = f32(blocks["fc1_w"])      # [depth, 2048, 512]
    fc1_b = f32(blocks["fc1_b"])
    fc2_w = f32(blocks["fc2_w"])      # [depth, 512, 2048]
    fc2_b = f32(blocks["fc2_b"])

    def chunkT(w):
        # [depth, O, Din] -> [depth, 128, Din//128, O]  (feature-major lhsT)
        d, O, Di = w.shape
        return np.ascontiguousarray(
            w.transpose(0, 2, 1).reshape(d, Di // 128, 128, O).transpose(0, 2, 1, 3))

    def colchunk(v):
        # [depth, Dout] -> [depth, 128, Dout//128]
        d, O = v.shape
        return np.ascontiguousarray(v.reshape(d, O // 128, 128).transpose(0, 2, 1))

    shared = {
        "tok": f32(p["tok"]),
        "posT": np.ascontiguousarray(
            f32(p["pos"]).T.reshape(4, 128, N).transpose(1, 0, 2)),
        "wqk": chunkT(qkv_w[:, :1024, :]),
        "wv": chunkT(qkv_w[:, 1024:, :]),
        "wproj": chunkT(proj_w),
        "wfc1": chunkT(fc1_w),
        "wfc2": chunkT(fc2_w),
        "qkb": colchunk(qkv_b[:, :1024]),
        "vb": colchunk(qkv_b[:, 1024:]),
        "projb": colchunk(proj_b),
        "fc1b": colchunk(fc1_b),
        "fc2b": colchunk(fc2_b),
        "ln1w": colchunk(f32(blocks["ln1_w"])),
        "ln1b": colchunk(f32(blocks["ln1_b"])),
        "ln2w": colchunk(f32(blocks["ln2_w"])),
        "ln2b": colchunk(f32(blocks["ln2_b"])),
        "normw": np.ascontiguousarray(f32(p["norm_w"]).reshape(4, 128).T),
        "normb": np.ascontiguousarray(f32(p["norm_b"]).reshape(4, 128).T),
    }
    headmap = {
        "sk": ("sk1_w", "sk1_b", "sk2_w", "sk2_b"),
        "em": ("em1_w", "em1_b", "em2_w", "em2_b"),
        "co": ("co1_w", "co1_b", "co2_w", "co2_b"),
        "cr": ("cr1_w", "cr1_b", "cr2_w", "cr2_b"),
        "wi": ("wi1_w", "wi1_b", "wi2_w", "wi2_b"),
        "im": ("im1_w", "im1_b", "im2_w", "im2_b"),
    }
    for nm, (w1k, b1k, w2k, b2k) in headmap.items():
        w1 = f32(p[w1k])  # [O1, 512]
        w2 = f32(p[w2k])  # [O2, O1]
        b1 = f32(p[b1k])
        b2 = f32(p[b2k])
        O1 = w1.shape[0]
        O2 = w2.shape[0]
        shared[f"{nm}1w"] = np.ascontiguousarray(
            w1.T.reshape(4, 128, O1).transpose(1, 0, 2))
        shared[f"{nm}1b"] = np.ascontiguousarray(b1.reshape(O1 // 128, 128).T)
        O2p = O2 + (O2 % 2)
        w2p = np.zeros((O2p, O1), np.float32)
        w2p[:O2] = w2
        shared[f"{nm}2w"] = np.ascontiguousarray(
            w2p.T.reshape(O1 // 128, 128, O2p).transpose(1, 0, 2))
        shared[f"{nm}2bt"] = np.ascontiguousarray(
            np.broadcast_to(b2[None, :], (BC, O2)))

    skill = f32(p["skill"])
    emo = f32(p["emo"])
    in_maps = []
    for core in range(NCORES):
        bs = slice(core * BC, (core + 1) * BC)
        skemo = skill[skill_ids[bs]] + emo[emotion_ids[bs]]  # [BC, 512]
        m = dict(shared)
        m["xids"] = np.ascontiguousarray(
            x[bs].reshape(BC * 4, 128))
        m["skemoT"] = np.ascontiguousarray(
            skemo.T.reshape(4, 128, BC).transpose(1, 0, 2))
        in_maps.append(m)
    return in_maps


def kernel(x, skill_ids, emotion_ids, params):
    global _PROG, LAST_RUN
    from concourse.bass_utils import run_bass_kernel_spmd

    if _PROG is None:
        _PROG = _build_program(DEPTH)
    nc = _PROG

    in_maps = _prep_inputs(x, skill_ids, emotion_ids, params, DEPTH)
    trace = os.environ.get("KERNEL_TRACE", "") == "1"
    res = run_bass_kernel_spmd(nc, in_maps, core_ids=list(range(NCORES)),
                               trace=trace)
    LAST_RUN = res

    pooled = np.concatenate([res.results[c]["pooled"] for c in range(NCORES)], 0)
    skills = np.concatenate([res.results[c]["skills"] for c in range(NCORES)], 0)
    emotions = np.concatenate([res.results[c]["emotions"] for c in range(NCORES)], 0)
    consc = np.concatenate([res.results[c]["consc"] for c in range(NCORES)], 0)
    creat = np.concatenate([res.results[c]["creat"] for c in range(NCORES)], 0)
    wisdom = np.concatenate([res.results[c]["wisdom"] for c in range(NCORES)], 0)
    improv = np.concatenate([res.results[c]["improv"] for c in range(NCORES)], 0)
    return (pooled, skills, emotions, consc, creat, wisdom, improv)
